# revision 1
# baseline (speedup 1.0000x reference)
"""LocalMHSA2D Trainium2 kernel: window (8x8) multi-head self-attention.

Full inputs -> shard batch B=8 across 8 NeuronCores -> full output.

Per-core dataflow (x_b: [256, 224, 224] f32, channels-first):
  - 28 slabs of 8 pixel rows (= one row of 28 windows each).
  - QKV projection as channel-major matmuls (contraction over C on partitions),
    fp32r on the PE at 1 cycle/row; evacuate q,k,v to SBUF as bf16.
  - Per window-pair attention:
      logits[s,t] per head via 32x64-tiled matmuls (4-way row / 2-way col
      concurrency on the PE array), exp on ACT (fused 1/sqrt(d) scale),
      row-sums + reciprocal + normalize on DVE, P^T via PE identity-matmul
      transposes, v^T via X-bar DMA transpose (bf16), AV via 64x32-tiled
      matmuls, all PSUM tiles bank-disjoint per PE row-tile group.
  - Out-projection (bf16->f32 psum) + bias, written back in spatial order so
    the slab store DMA is contiguous.

This walrus build rejects instructions carrying >1 semaphore wait
("Too many sync wait commands"), so a post-pass splits excess waits
onto same-engine no-ops.
"""

import numpy as np
import ml_dtypes

# ---- tunables -------------------------------------------------------------
PROJ_F32R = True          # fp32r (1 cyc/row) vs fp32 (4 cyc/row) for projections
N_SLAB = 7                # slabs (8-row strips) per NEFF invocation; best fresh-process first-call wall
CORES = 8

_CACHE = {}


def _build(nslab):
    import concourse.bass as bass
    import concourse.mybir as mybir
    import concourse.tile as tile
    from concourse.masks import make_identity
    from concourse.bass import ds

    f32 = mybir.dt.float32
    f32r = mybir.dt.float32r
    bf16 = mybir.dt.bfloat16

    PF = f32r if PROJ_F32R else f32

    def r32(ap):
        return ap

    nc = bass.Bass()
    HH = nslab * 8
    x_d = nc.dram_tensor("x", [256, HH, 224], f32, kind="ExternalInput")
    wq_d = nc.dram_tensor("wqkvT", [256, 768], f32, kind="ExternalInput")
    wo_d = nc.dram_tensor("woutT", [256, 256], f32, kind="ExternalInput")
    bq_d = nc.dram_tensor("bqkv", [128, 6], f32, kind="ExternalInput")
    bo_d = nc.dram_tensor("bout", [128, 2], f32, kind="ExternalInput")
    y_d = nc.dram_tensor("y", [256, HH, 224], f32, kind="ExternalOutput")

    # [128 parts, chunk, ...] views of dram tensors
    x_v = x_d.rearrange("(cc p) hh w -> p cc hh w", p=128)
    y_v = y_d.rearrange("(cc p) hh w -> p cc hh w", p=128)
    wq_v = wq_d.rearrange("(cc p) e -> p cc e", p=128)
    wo_v = wo_d.rearrange("(cc p) e -> p cc e", p=128)
    if PROJ_F32R:
        x_v = x_v.bitcast(f32r)
        wq_v = wq_v.bitcast(f32r)
        wo_v = wo_v.bitcast(f32r)

    EXP_SCALE = float(1.0 / np.sqrt(32.0))

    with tile.TileContext(nc) as tc:
        with (
            tc.tile_pool(name="static", bufs=1) as static,
            tc.tile_pool(name="xin", bufs=2) as xpool,
            tc.tile_pool(name="qkv", bufs=2) as qkvpool,
            tc.tile_pool(name="osb", bufs=2) as opool_sb,
            tc.tile_pool(name="ysb", bufs=2) as ypool,
            tc.tile_pool(name="psb", bufs=3) as ppool,
            tc.tile_pool(name="ptsb", bufs=3) as ptpool_sb,
            tc.tile_pool(name="vtsb", bufs=3) as vtpool,
            tc.tile_pool(name="vdup", bufs=3) as vdpool,
            tc.tile_pool(name="small", bufs=4) as spool,
            tc.tile_pool(name="projps", bufs=2, space="PSUM") as projps,
            tc.tile_pool(name="attnps", bufs=1, space="PSUM") as attnps,
            tc.tile_pool(name="ptps", bufs=1, space="PSUM") as ptps,
        ):
            # ---- static tiles ----
            wq_sb = static.tile([128, 2, 768], PF)
            wo_sb = static.tile([128, 2, 256], PF)
            bq_sb = static.tile([128, 6], f32)
            bo_sb = static.tile([128, 2], f32)
            ident = static.tile([128, 64], bf16)
            nc.sync.dma_start(out=wq_sb, in_=wq_v)
            nc.sync.dma_start(out=wo_sb, in_=wo_v)
            nc.sync.dma_start(out=bq_sb, in_=bq_d[:, :])
            nc.sync.dma_start(out=bo_sb, in_=bo_d[:, :])
            make_identity(nc, ident[0:64, :])
            make_identity(nc, ident[64:128, :])

            for i in range(nslab):
                # ---- load slab: [128, chunk, 8 rows, 224] ----
                x_sb = xpool.tile([128, 2, 8, 224], PF)
                nc.gpsimd.dma_start(out=x_sb, in_=x_v[:, :, ds(i * 8, 8), :])

                q_sb = qkvpool.tile([128, 2, 1792], bf16, tag="q")
                k_sb = qkvpool.tile([128, 2, 1792], bf16, tag="k")
                v_sb = qkvpool.tile([128, 2, 1792], bf16, tag="v")
                o_sb = opool_sb.tile([128, 2, 1792], PF)
                y_sb = ypool.tile([128, 2, 8, 224], f32)

                # ---- QKV projection, groups of 7 windows (448 tokens) ----
                for g in range(4):
                    xg = [
                        x_sb[:, ch].rearrange("p h (G j w) -> p G j h w", j=7, w=8)[:, g]
                        for ch in range(2)
                    ]
                    for eb in range(6):
                        ps = projps.tile([128, 448], f32, tag="proj")
                        nc.tensor.matmul(
                            out=ps, lhsT=r32(wq_sb[:, 0, 128 * eb : 128 * eb + 128]),
                            rhs=r32(xg[0]), start=True, stop=False,
                        )
                        nc.tensor.matmul(
                            out=ps, lhsT=r32(wq_sb[:, 1, 128 * eb : 128 * eb + 128]),
                            rhs=r32(xg[1]), start=False, stop=True,
                        )
                        dest = (q_sb, q_sb, k_sb, k_sb, v_sb, v_sb)[eb]
                        dst = dest[:, eb % 2, 448 * g : 448 * g + 448]
                        if eb in (0, 2):
                            nc.vector.tensor_scalar_add(
                                out=dst, in0=ps, scalar1=bq_sb[:, eb : eb + 1]
                            )
                        else:
                            nc.scalar.activation(
                                out=dst, in_=ps,
                                func=mybir.ActivationFunctionType.Identity,
                                bias=bq_sb[:, eb : eb + 1], scale=1.0,
                            )

                # ---- attention: 14 window pairs, superblocks of 2 pairs ----
                for sb_i in range(7):
                    SB = attnps.tile([128, 4, 512], f32)  # 4 banks: logits + o
                    PT_ps0 = ptps.tile([128, 2, 4, 64], bf16, tag="pt0")
                    PT_ps1 = ptps.tile([128, 2, 4, 64], bf16, tag="pt1")
                    PT_ps = [PT_ps0, PT_ps1]
                    for q_i in range(2):
                        p = 2 * sb_i + q_i
                        # logits[s, t] per head h = j + 4*hi
                        for h in range(8):
                            j, hi = h % 4, h // 4
                            for wi in range(2):
                                w = 2 * p + wi
                                nc.tensor.matmul(
                                    out=SB[64 * wi : 64 * wi + 64, j,
                                           128 * q_i + 64 * hi : 128 * q_i + 64 * hi + 64],
                                    lhsT=q_sb[32 * j : 32 * j + 32, hi, 64 * w : 64 * w + 64],
                                    rhs=k_sb[32 * j : 32 * j + 32, hi, 64 * w : 64 * w + 64],
                                    start=True, stop=True,
                                    tile_position=(32 * j, 64 * wi),
                                )
                        # P = exp(logits / sqrt(d)); free col = 128*j + 64*hi + t
                        P = ppool.tile([128, 512], bf16)
                        nc.scalar.activation(
                            out=P[:].rearrange("p (a b) -> p a b", a=4),
                            in_=SB[:, :, 128 * q_i : 128 * q_i + 128],
                            func=mybir.ActivationFunctionType.Exp, scale=EXP_SCALE,
                        )
                        # row-sums over t, reciprocal, expand (gpsimd), normalize
                        sums = spool.tile([128, 8], f32, tag="sums")
                        rsum = spool.tile([128, 8], f32, tag="rsum")
                        rsx = spool.tile([128, 512], bf16, tag="rsx")
                        nc.vector.tensor_reduce(
                            out=sums, in_=P[:].rearrange("p (c t) -> p c t", t=64),
                            axis=mybir.AxisListType.X, op=mybir.AluOpType.add,
                        )
                        nc.vector.reciprocal(out=rsum, in_=sums)
                        rs = rsum[:]
                        rs_b = bass.AP(rs.tensor, rs.offset, [rs.ap[0], [1, 8], [0, 64]])
                        nc.gpsimd.tensor_copy(out=rsx, in_=rs_b)
                        nc.vector.tensor_mul(out=P, in0=P, in1=rsx)

                        # P^T via PE transpose: per (wi, j) -> [2 heads x 64t, 64s]
                        for wi in range(2):
                            for j in range(4):
                                nc.tensor.transpose(
                                    out=PT_ps[wi][:, q_i, j, :],
                                    in_=P[64 * wi : 64 * wi + 64, 128 * j : 128 * j + 128],
                                    identity=ident[64 * wi : 64 * wi + 64, :],
                                    tile_position=(64 * wi, 0),
                                )
                        PT = ptpool_sb.tile([128, 2, 4, 64], bf16)
                        nc.vector.tensor_copy(out=PT[:, 0], in_=PT_ps[0][:, q_i])
                        nc.scalar.copy(out=PT[:, 1], in_=PT_ps[1][:, q_i])

                        # v^T via dup-copy + X-bar DMA transpose (t replicated)
                        vd = vdpool.tile([128, 4, 128], bf16)
                        vt = vtpool.tile([128, 2, 2, 128], bf16)  # [t-rep, wi, ch, c]
                        for wi in range(2):
                            w = 2 * p + wi
                            for ch in range(2):
                                a = v_sb[:, ch, 64 * w : 64 * w + 64]
                                a_dup = bass.AP(a.tensor, a.offset, [a.ap[0], [0, 2]] + list(a.ap[1:]))
                                nc.gpsimd.tensor_copy(out=vd[:, 2 * wi + ch], in_=a_dup)
                                nc.sync.dma_start(
                                    out=vt[:, wi, ch], in_=vd[:, 2 * wi + ch], transpose=True
                                )

                        # AV: o[d, s] per head into SB cols 256+: bank 2*hi
                        for h in range(8):
                            j, hi = h % 4, h // 4
                            for wi in range(2):
                                nc.tensor.matmul(
                                    out=SB[32 * j : 32 * j + 32, 2 * hi,
                                           256 + 128 * q_i + 64 * wi : 256 + 128 * q_i + 64 * wi + 64],
                                    lhsT=vt[64 * hi : 64 * hi + 64, wi, hi, 32 * j : 32 * j + 32],
                                    rhs=PT[64 * hi : 64 * hi + 64, wi, j, :],
                                    start=True, stop=True,
                                    tile_position=(64 * hi, 32 * j),
                                )
                        # evacuate o (channel-major: chunk hi = heads 4*hi..)
                        for hi in range(2):
                            src = SB[:, 2 * hi, 256 + 128 * q_i : 256 + 128 * q_i + 128]
                            dst = o_sb[:, hi, 128 * p : 128 * p + 128]
                            if hi == 0:
                                nc.scalar.copy(out=dst, in_=src)
                            else:
                                nc.vector.tensor_copy(out=dst, in_=src)

                # ---- out-projection (bf16 o? -> fp32(r) matmul over C) ----
                for g in range(4):
                    yg = [
                        y_sb[:, ob].rearrange("p h (G j w) -> p G j h w", j=7, w=8)[:, g]
                        for ob in range(2)
                    ]
                    for ob in range(2):
                        ps = projps.tile([128, 448], f32, tag="proj")
                        nc.tensor.matmul(
                            out=ps, lhsT=r32(wo_sb[:, 0, 128 * ob : 128 * ob + 128]),
                            rhs=r32(o_sb[:, 0, 448 * g : 448 * g + 448]),
                            start=True, stop=False,
                        )
                        nc.tensor.matmul(
                            out=ps, lhsT=r32(wo_sb[:, 1, 128 * ob : 128 * ob + 128]),
                            rhs=r32(o_sb[:, 1, 448 * g : 448 * g + 448]),
                            start=False, stop=True,
                        )
                        psv = ps[:].rearrange("p (j h w) -> p j h w", h=8, w=8)
                        if (g + ob) % 2 == 0:
                            nc.vector.tensor_scalar_add(
                                out=yg[ob], in0=psv, scalar1=bo_sb[:, ob : ob + 1]
                            )
                        else:
                            nc.scalar.activation(
                                out=yg[ob], in_=psv,
                                func=mybir.ActivationFunctionType.Identity,
                                bias=bo_sb[:, ob : ob + 1], scale=1.0,
                            )

                nc.gpsimd.dma_start(out=y_v[:, :, ds(i * 8, 8), :], in_=y_sb)

    _split_excess_waits(nc)
    return nc


def _split_excess_waits(nc, limit=1):
    import concourse.mybir as mybir

    n_new = 0
    for f in nc.m.functions:
        for bb in f.blocks:
            insts = bb.instructions
            i = 0
            while i < len(insts):
                inst = insts[i]
                si = inst.sync_info
                if si is not None and si.on_wait and len(si.on_wait) > limit:
                    waits = list(si.on_wait)
                    si.on_wait = waits[:limit]
                    rest = waits[limit:]
                    for k in range(0, len(rest), limit):
                        nop = mybir.InstNoOp(name=f"{inst.name}-wsplit{k}", ins=[], outs=[])
                        nop.engine = inst.engine
                        nop.sync_info = mybir.SyncInfo(on_wait=rest[k : k + limit], on_update=[])
                        insts.insert(i, nop)
                        n_new += 1
                        i += 1
                i += 1
    return n_new


def _get_nc(nslab):
    if nslab not in _CACHE:
        _CACHE[nslab] = _build(nslab)
    return _CACHE[nslab]


def _host_prep(w_in, b_in, w_out, b_out):
    f = np.float32
    wqkvT = np.ascontiguousarray(w_in.astype(f).T)          # [256, 768]
    woutT = np.ascontiguousarray(w_out.astype(f).T)         # [256, 256]
    bqkv = np.ascontiguousarray(b_in.astype(f).reshape(6, 128).T)  # [128, 6]
    bout = np.ascontiguousarray(b_out.astype(f).reshape(2, 128).T)  # [128, 2]
    return wqkvT, woutT, bqkv, bout


def kernel(x, w_in, b_in, w_out, b_out, _nslab=N_SLAB, _trace=False):
    from concourse.bass_utils import run_bass_kernel_spmd

    x = np.asarray(x, dtype=np.float32)
    B = x.shape[0]
    wqkvT, woutT, bqkv, bout = _host_prep(
        np.asarray(w_in), np.asarray(b_in), np.asarray(w_out), np.asarray(b_out)
    )
    nc = _get_nc(_nslab)
    H = x.shape[2]
    rows = _nslab * 8
    n_chunks = (H + rows - 1) // rows
    y = np.empty_like(x)
    for c in range(n_chunks):
        r0 = c * rows
        in_maps = []
        for b in range(CORES):
            xb = x[b % B]
            in_maps.append({
                "x": np.ascontiguousarray(xb[:, r0 : r0 + rows, :]),
                "wqkvT": wqkvT, "woutT": woutT, "bqkv": bqkv, "bout": bout,
            })
        res = run_bass_kernel_spmd(
            nc, in_maps, core_ids=list(range(CORES)), trace=_trace
        )
        for b in range(B):
            y[b, :, r0 : r0 + rows, :] = res.results[b]["y"]
        kernel.last_result = res
    return y



# revision 3
# speedup vs baseline: 13443.0797x; 13443.0797x over previous
"""LocalMHSA2D Trainium2 kernel: window (8x8) multi-head self-attention.

Full inputs -> shard batch B=8 across 8 NeuronCores -> full output.

Per-core dataflow (x_b: [256, 224, 224] f32, channels-first):
  - 28 slabs of 8 pixel rows (= one row of 28 windows each).
  - QKV projection as channel-major matmuls (contraction over C on partitions),
    fp32r on the PE at 1 cycle/row; evacuate q,k,v to SBUF as bf16.
  - Per window-pair attention:
      logits[s,t] per head via 32x64-tiled matmuls (4-way row / 2-way col
      concurrency on the PE array), exp on ACT (fused 1/sqrt(d) scale),
      row-sums + reciprocal + normalize on DVE, P^T via PE identity-matmul
      transposes, v^T via X-bar DMA transpose (bf16), AV via 64x32-tiled
      matmuls, all PSUM tiles bank-disjoint per PE row-tile group.
  - Out-projection (bf16->f32 psum) + bias, written back in spatial order so
    the slab store DMA is contiguous.

This walrus build rejects instructions carrying >1 semaphore wait
("Too many sync wait commands"), so a post-pass splits excess waits
onto same-engine no-ops.
"""

import numpy as np
import ml_dtypes

# ---- tunables -------------------------------------------------------------
PROJ_F32R = True          # fp32r (1 cyc/row) vs fp32 (4 cyc/row) for projections
N_SLAB = 7                # slabs (8-row strips) per NEFF invocation; best fresh-process first-call wall
CORES = 8

_CACHE = {}


def _install_ntff_hook():
    """Register the axon NTFF profile hook if the image's antenv lacks it.

    concourse.bass_utils fetches the hook via ``antenv.axon_hooks``; on this
    image that module is absent, so ``trace=True`` can't produce a device
    exec time. The boot module ships the ctypes-based hook constructor, so
    provide the missing registry in-process.
    """
    import sys
    try:
        from antenv.axon_hooks import get_axon_ntff_profile_hook  # noqa: F401
        return  # real module present
    except ImportError:
        pass
    try:
        import types
        from trn_agent_boot.trn_boot import _ntff_profile_via_ctypes
        hook = _ntff_profile_via_ctypes("/opt/axon/libaxon_pjrt.so")
        if hook is None:
            return
        mod = types.ModuleType("antenv.axon_hooks")
        _h = [hook]
        mod.set_axon_ntff_profile_hook = lambda h: _h.__setitem__(0, h)
        mod.get_axon_ntff_profile_hook = lambda: _h[0]
        sys.modules["antenv.axon_hooks"] = mod
    except Exception:
        pass


def _build(nslab):
    import concourse.bass as bass
    import concourse.mybir as mybir
    import concourse.tile as tile
    from concourse.masks import make_identity
    from concourse.bass import ds

    f32 = mybir.dt.float32
    f32r = mybir.dt.float32r
    bf16 = mybir.dt.bfloat16

    PF = f32r if PROJ_F32R else f32

    def r32(ap):
        return ap

    nc = bass.Bass()
    HH = nslab * 8
    x_d = nc.dram_tensor("x", [256, HH, 224], f32, kind="ExternalInput")
    wq_d = nc.dram_tensor("wqkvT", [256, 768], f32, kind="ExternalInput")
    wo_d = nc.dram_tensor("woutT", [256, 256], f32, kind="ExternalInput")
    bq_d = nc.dram_tensor("bqkv", [128, 6], f32, kind="ExternalInput")
    bo_d = nc.dram_tensor("bout", [128, 2], f32, kind="ExternalInput")
    y_d = nc.dram_tensor("y", [256, HH, 224], f32, kind="ExternalOutput")

    # [128 parts, chunk, ...] views of dram tensors
    x_v = x_d.rearrange("(cc p) hh w -> p cc hh w", p=128)
    y_v = y_d.rearrange("(cc p) hh w -> p cc hh w", p=128)
    wq_v = wq_d.rearrange("(cc p) e -> p cc e", p=128)
    wo_v = wo_d.rearrange("(cc p) e -> p cc e", p=128)
    if PROJ_F32R:
        x_v = x_v.bitcast(f32r)
        wq_v = wq_v.bitcast(f32r)
        wo_v = wo_v.bitcast(f32r)

    EXP_SCALE = float(1.0 / np.sqrt(32.0))

    with tile.TileContext(nc) as tc:
        with (
            tc.tile_pool(name="static", bufs=1) as static,
            tc.tile_pool(name="xin", bufs=2) as xpool,
            tc.tile_pool(name="qkv", bufs=2) as qkvpool,
            tc.tile_pool(name="osb", bufs=2) as opool_sb,
            tc.tile_pool(name="ysb", bufs=2) as ypool,
            tc.tile_pool(name="psb", bufs=3) as ppool,
            tc.tile_pool(name="ptsb", bufs=3) as ptpool_sb,
            tc.tile_pool(name="vtsb", bufs=3) as vtpool,
            tc.tile_pool(name="vdup", bufs=3) as vdpool,
            tc.tile_pool(name="small", bufs=4) as spool,
            tc.tile_pool(name="projps", bufs=2, space="PSUM") as projps,
            tc.tile_pool(name="attnps", bufs=1, space="PSUM") as attnps,
            tc.tile_pool(name="ptps", bufs=1, space="PSUM") as ptps,
        ):
            # ---- static tiles ----
            wq_sb = static.tile([128, 2, 768], PF)
            wo_sb = static.tile([128, 2, 256], PF)
            bq_sb = static.tile([128, 6], f32)
            bo_sb = static.tile([128, 2], f32)
            ident = static.tile([128, 64], bf16)
            nc.sync.dma_start(out=wq_sb, in_=wq_v)
            nc.sync.dma_start(out=wo_sb, in_=wo_v)
            nc.sync.dma_start(out=bq_sb, in_=bq_d[:, :])
            nc.sync.dma_start(out=bo_sb, in_=bo_d[:, :])
            make_identity(nc, ident[0:64, :])
            make_identity(nc, ident[64:128, :])

            for i in range(nslab):
                # ---- load slab: [128, chunk, 8 rows, 224] ----
                x_sb = xpool.tile([128, 2, 8, 224], PF)
                nc.gpsimd.dma_start(out=x_sb, in_=x_v[:, :, ds(i * 8, 8), :])

                q_sb = qkvpool.tile([128, 2, 1792], bf16, tag="q")
                k_sb = qkvpool.tile([128, 2, 1792], bf16, tag="k")
                v_sb = qkvpool.tile([128, 2, 1792], bf16, tag="v")
                o_sb = opool_sb.tile([128, 2, 1792], PF)
                y_sb = ypool.tile([128, 2, 8, 224], f32)

                # ---- QKV projection, groups of 7 windows (448 tokens) ----
                for g in range(4):
                    xg = [
                        x_sb[:, ch].rearrange("p h (G j w) -> p G j h w", j=7, w=8)[:, g]
                        for ch in range(2)
                    ]
                    for eb in range(6):
                        ps = projps.tile([128, 448], f32, tag="proj")
                        nc.tensor.matmul(
                            out=ps, lhsT=r32(wq_sb[:, 0, 128 * eb : 128 * eb + 128]),
                            rhs=r32(xg[0]), start=True, stop=False,
                        )
                        nc.tensor.matmul(
                            out=ps, lhsT=r32(wq_sb[:, 1, 128 * eb : 128 * eb + 128]),
                            rhs=r32(xg[1]), start=False, stop=True,
                        )
                        dest = (q_sb, q_sb, k_sb, k_sb, v_sb, v_sb)[eb]
                        dst = dest[:, eb % 2, 448 * g : 448 * g + 448]
                        if eb in (0, 2):
                            nc.vector.tensor_scalar_add(
                                out=dst, in0=ps, scalar1=bq_sb[:, eb : eb + 1]
                            )
                        else:
                            nc.scalar.activation(
                                out=dst, in_=ps,
                                func=mybir.ActivationFunctionType.Identity,
                                bias=bq_sb[:, eb : eb + 1], scale=1.0,
                            )

                # ---- attention: 14 window pairs, superblocks of 2 pairs ----
                for sb_i in range(7):
                    SB = attnps.tile([128, 4, 512], f32)  # 4 banks: logits + o
                    PT_ps0 = ptps.tile([128, 2, 4, 64], bf16, tag="pt0")
                    PT_ps1 = ptps.tile([128, 2, 4, 64], bf16, tag="pt1")
                    PT_ps = [PT_ps0, PT_ps1]
                    for q_i in range(2):
                        p = 2 * sb_i + q_i
                        # logits[s, t] per head h = j + 4*hi
                        for h in range(8):
                            j, hi = h % 4, h // 4
                            for wi in range(2):
                                w = 2 * p + wi
                                nc.tensor.matmul(
                                    out=SB[64 * wi : 64 * wi + 64, j,
                                           128 * q_i + 64 * hi : 128 * q_i + 64 * hi + 64],
                                    lhsT=q_sb[32 * j : 32 * j + 32, hi, 64 * w : 64 * w + 64],
                                    rhs=k_sb[32 * j : 32 * j + 32, hi, 64 * w : 64 * w + 64],
                                    start=True, stop=True,
                                    tile_position=(32 * j, 64 * wi),
                                )
                        # P = exp(logits / sqrt(d)); free col = 128*j + 64*hi + t
                        P = ppool.tile([128, 512], bf16)
                        nc.scalar.activation(
                            out=P[:].rearrange("p (a b) -> p a b", a=4),
                            in_=SB[:, :, 128 * q_i : 128 * q_i + 128],
                            func=mybir.ActivationFunctionType.Exp, scale=EXP_SCALE,
                        )
                        # row-sums over t, reciprocal, expand (gpsimd), normalize
                        sums = spool.tile([128, 8], f32, tag="sums")
                        rsum = spool.tile([128, 8], f32, tag="rsum")
                        rsx = spool.tile([128, 512], bf16, tag="rsx")
                        nc.vector.tensor_reduce(
                            out=sums, in_=P[:].rearrange("p (c t) -> p c t", t=64),
                            axis=mybir.AxisListType.X, op=mybir.AluOpType.add,
                        )
                        nc.vector.reciprocal(out=rsum, in_=sums)
                        rs = rsum[:]
                        rs_b = bass.AP(rs.tensor, rs.offset, [rs.ap[0], [1, 8], [0, 64]])
                        nc.gpsimd.tensor_copy(out=rsx, in_=rs_b)
                        nc.vector.tensor_mul(out=P, in0=P, in1=rsx)

                        # P^T via PE transpose: per (wi, j) -> [2 heads x 64t, 64s]
                        for wi in range(2):
                            for j in range(4):
                                nc.tensor.transpose(
                                    out=PT_ps[wi][:, q_i, j, :],
                                    in_=P[64 * wi : 64 * wi + 64, 128 * j : 128 * j + 128],
                                    identity=ident[64 * wi : 64 * wi + 64, :],
                                    tile_position=(64 * wi, 0),
                                )
                        PT = ptpool_sb.tile([128, 2, 4, 64], bf16)
                        nc.vector.tensor_copy(out=PT[:, 0], in_=PT_ps[0][:, q_i])
                        nc.scalar.copy(out=PT[:, 1], in_=PT_ps[1][:, q_i])

                        # v^T via dup-copy + X-bar DMA transpose (t replicated)
                        vd = vdpool.tile([128, 4, 128], bf16)
                        vt = vtpool.tile([128, 2, 2, 128], bf16)  # [t-rep, wi, ch, c]
                        for wi in range(2):
                            w = 2 * p + wi
                            for ch in range(2):
                                a = v_sb[:, ch, 64 * w : 64 * w + 64]
                                a_dup = bass.AP(a.tensor, a.offset, [a.ap[0], [0, 2]] + list(a.ap[1:]))
                                nc.gpsimd.tensor_copy(out=vd[:, 2 * wi + ch], in_=a_dup)
                                nc.sync.dma_start(
                                    out=vt[:, wi, ch], in_=vd[:, 2 * wi + ch], transpose=True
                                )

                        # AV: o[d, s] per head into SB cols 256+: bank 2*hi
                        for h in range(8):
                            j, hi = h % 4, h // 4
                            for wi in range(2):
                                nc.tensor.matmul(
                                    out=SB[32 * j : 32 * j + 32, 2 * hi,
                                           256 + 128 * q_i + 64 * wi : 256 + 128 * q_i + 64 * wi + 64],
                                    lhsT=vt[64 * hi : 64 * hi + 64, wi, hi, 32 * j : 32 * j + 32],
                                    rhs=PT[64 * hi : 64 * hi + 64, wi, j, :],
                                    start=True, stop=True,
                                    tile_position=(64 * hi, 32 * j),
                                )
                        # evacuate o (channel-major: chunk hi = heads 4*hi..)
                        for hi in range(2):
                            src = SB[:, 2 * hi, 256 + 128 * q_i : 256 + 128 * q_i + 128]
                            dst = o_sb[:, hi, 128 * p : 128 * p + 128]
                            if hi == 0:
                                nc.scalar.copy(out=dst, in_=src)
                            else:
                                nc.vector.tensor_copy(out=dst, in_=src)

                # ---- out-projection (bf16 o? -> fp32(r) matmul over C) ----
                for g in range(4):
                    yg = [
                        y_sb[:, ob].rearrange("p h (G j w) -> p G j h w", j=7, w=8)[:, g]
                        for ob in range(2)
                    ]
                    for ob in range(2):
                        ps = projps.tile([128, 448], f32, tag="proj")
                        nc.tensor.matmul(
                            out=ps, lhsT=r32(wo_sb[:, 0, 128 * ob : 128 * ob + 128]),
                            rhs=r32(o_sb[:, 0, 448 * g : 448 * g + 448]),
                            start=True, stop=False,
                        )
                        nc.tensor.matmul(
                            out=ps, lhsT=r32(wo_sb[:, 1, 128 * ob : 128 * ob + 128]),
                            rhs=r32(o_sb[:, 1, 448 * g : 448 * g + 448]),
                            start=False, stop=True,
                        )
                        psv = ps[:].rearrange("p (j h w) -> p j h w", h=8, w=8)
                        if (g + ob) % 2 == 0:
                            nc.vector.tensor_scalar_add(
                                out=yg[ob], in0=psv, scalar1=bo_sb[:, ob : ob + 1]
                            )
                        else:
                            nc.scalar.activation(
                                out=yg[ob], in_=psv,
                                func=mybir.ActivationFunctionType.Identity,
                                bias=bo_sb[:, ob : ob + 1], scale=1.0,
                            )

                nc.gpsimd.dma_start(out=y_v[:, :, ds(i * 8, 8), :], in_=y_sb)

    _split_excess_waits(nc)
    return nc


def _split_excess_waits(nc, limit=1):
    import concourse.mybir as mybir

    n_new = 0
    for f in nc.m.functions:
        for bb in f.blocks:
            insts = bb.instructions
            i = 0
            while i < len(insts):
                inst = insts[i]
                si = inst.sync_info
                if si is not None and si.on_wait and len(si.on_wait) > limit:
                    waits = list(si.on_wait)
                    si.on_wait = waits[:limit]
                    rest = waits[limit:]
                    for k in range(0, len(rest), limit):
                        nop = mybir.InstNoOp(name=f"{inst.name}-wsplit{k}", ins=[], outs=[])
                        nop.engine = inst.engine
                        nop.sync_info = mybir.SyncInfo(on_wait=rest[k : k + limit], on_update=[])
                        insts.insert(i, nop)
                        n_new += 1
                        i += 1
                i += 1
    return n_new


def _get_nc(nslab):
    if nslab not in _CACHE:
        _CACHE[nslab] = _build(nslab)
    return _CACHE[nslab]


def _host_prep(w_in, b_in, w_out, b_out):
    f = np.float32
    wqkvT = np.ascontiguousarray(w_in.astype(f).T)          # [256, 768]
    woutT = np.ascontiguousarray(w_out.astype(f).T)         # [256, 256]
    bqkv = np.ascontiguousarray(b_in.astype(f).reshape(6, 128).T)  # [128, 6]
    bout = np.ascontiguousarray(b_out.astype(f).reshape(2, 128).T)  # [128, 2]
    return wqkvT, woutT, bqkv, bout


def kernel(x, w_in, b_in, w_out, b_out, _nslab=N_SLAB, _trace=False):
    if _trace:
        _install_ntff_hook()
    from concourse.bass_utils import run_bass_kernel_spmd

    x = np.asarray(x, dtype=np.float32)
    B = x.shape[0]
    wqkvT, woutT, bqkv, bout = _host_prep(
        np.asarray(w_in), np.asarray(b_in), np.asarray(w_out), np.asarray(b_out)
    )
    nc = _get_nc(_nslab)
    H = x.shape[2]
    rows = _nslab * 8
    n_chunks = (H + rows - 1) // rows
    y = np.empty_like(x)
    kernel.total_exec_time_ns = 0
    for c in range(n_chunks):
        r0 = c * rows
        in_maps = []
        for b in range(CORES):
            xb = x[b % B]
            in_maps.append({
                "x": np.ascontiguousarray(xb[:, r0 : r0 + rows, :]),
                "wqkvT": wqkvT, "woutT": woutT, "bqkv": bqkv, "bout": bout,
            })
        res = run_bass_kernel_spmd(
            nc, in_maps, core_ids=list(range(CORES)), trace=_trace
        )
        for b in range(B):
            y[b, :, r0 : r0 + rows, :] = res.results[b]["y"]
        kernel.last_result = res
        if res.exec_time_ns is not None:
            kernel.total_exec_time_ns += res.exec_time_ns
    if not kernel.total_exec_time_ns:
        kernel.total_exec_time_ns = None
    return y



# revision 4
# speedup vs baseline: 15997.8313x; 1.1900x over previous
"""LocalMHSA2D Trainium2 kernel v2: window (8x8) multi-head self-attention.

Full inputs -> shard batch B=8 across 8 NeuronCores -> full output.

Per-core dataflow (x_b: [256, 224, 224] bf16, channels-first), one hardware
loop over 28 slabs of 8 pixel rows (28 windows each, 14 window pairs):

  - Q,K projection channel-major (contraction over C on partitions), bf16.
  - V projected directly into transposed layout (tokens on partitions) by
    swapping matmul operands: lhsT = x-tile [cin, 128 tok], rhs = w_v.
    w_v is augmented to 264 columns: per head 32 value dims + 1 ones column
    (via the bias row), which later yields softmax denominators for free.
  - Attention per window pair, all layouts chosen so no P/V transposes are
    needed:
      logits^T directly via lhsT=k, rhs=q -> PSUM [128 (wi,t), (j,hi,s)];
      PT = exp(scale*logits^T) on ACT -> SBUF bf16;
      AV: lhsT = PT slice [t, s], rhs = vt [t, 33] -> o^T [(wi,s), (h, d|sum)]
      with the softmax denominator interleaved as column 33h+32;
      normalize o^T with 8 per-head tensor_scalar_mul (factor = per-partition
      reciprocal of the denominator column);
      2 PE transposes (128x128) restore channel-major o for the out-proj.
  - Out-projection + bias, written back in spatial order, y stored as bf16.

This walrus build rejects instructions carrying >1 semaphore wait
("Too many sync wait commands"), so a post-pass splits excess waits
onto same-engine no-ops.
"""

import numpy as np
import ml_dtypes

CORES = 8
N_SLABS = 28

_CACHE = {}


def _install_ntff_hook():
    """Register the axon NTFF profile hook if the image's antenv lacks it.

    concourse.bass_utils fetches the hook via ``antenv.axon_hooks``; on this
    image that module is absent, so ``trace=True`` can't produce a device
    exec time. The boot module ships the ctypes-based hook constructor, so
    provide the missing registry in-process.
    """
    import sys
    try:
        from antenv.axon_hooks import get_axon_ntff_profile_hook  # noqa: F401
        return
    except ImportError:
        pass
    try:
        import types
        from trn_agent_boot.trn_boot import _ntff_profile_via_ctypes
        hook = _ntff_profile_via_ctypes("/opt/axon/libaxon_pjrt.so")
        if hook is None:
            return
        mod = types.ModuleType("antenv.axon_hooks")
        _h = [hook]
        mod.set_axon_ntff_profile_hook = lambda h: _h.__setitem__(0, h)
        mod.get_axon_ntff_profile_hook = lambda: _h[0]
        sys.modules["antenv.axon_hooks"] = mod
    except Exception:
        pass


def _build(n_slabs, split_waits=True):
    import concourse.bass as bass
    import concourse.mybir as mybir
    import concourse.tile as tile
    from concourse.masks import make_identity
    from concourse.bass import ds

    f32 = mybir.dt.float32
    bf16 = mybir.dt.bfloat16
    AF = mybir.ActivationFunctionType

    nc = bass.Bass()
    HH = n_slabs * 8
    x_d = nc.dram_tensor("x", [256, HH, 224], bf16, kind="ExternalInput")
    wqk_d = nc.dram_tensor("wqkT", [256, 512], bf16, kind="ExternalInput")
    wv_d = nc.dram_tensor("wvT", [256, 264], bf16, kind="ExternalInput")
    wo_d = nc.dram_tensor("woT", [256, 256], bf16, kind="ExternalInput")
    bqk_d = nc.dram_tensor("bqk", [128, 4], f32, kind="ExternalInput")
    bv_d = nc.dram_tensor("bv", [1, 264], bf16, kind="ExternalInput")
    bo_d = nc.dram_tensor("bo", [128, 2], f32, kind="ExternalInput")
    y_d = nc.dram_tensor("y", [256, HH, 224], bf16, kind="ExternalOutput")

    x_v = x_d.rearrange("(cc p) hh w -> p cc hh w", p=128)
    y_v = y_d.rearrange("(cc p) hh w -> p cc hh w", p=128)
    wqk_v = wqk_d.rearrange("(cc p) e -> p cc e", p=128)
    wv_v = wv_d.rearrange("(cc p) e -> p cc e", p=128)
    wo_v = wo_d.rearrange("(cc p) e -> p cc e", p=128)

    EXP_SCALE = float(1.0 / np.sqrt(32.0))

    with tile.TileContext(nc) as tc:
        with (
            tc.tile_pool(name="static", bufs=1) as static,
            tc.tile_pool(name="xin", bufs=2) as xpool,
            tc.tile_pool(name="qk", bufs=2) as qkpool,
            tc.tile_pool(name="vt", bufs=4) as vtpool,
            tc.tile_pool(name="pt", bufs=3) as ptpool,
            tc.tile_pool(name="otn", bufs=3) as otnpool,
            tc.tile_pool(name="osb", bufs=2) as opool,
            tc.tile_pool(name="ysb", bufs=2) as ypool,
            tc.tile_pool(name="small", bufs=4) as spool,
            tc.tile_pool(name="projps", bufs=2, space="PSUM") as projps,
            tc.tile_pool(name="ltps", bufs=1, space="PSUM") as ltps,
            tc.tile_pool(name="otps", bufs=1, space="PSUM") as otps,
            tc.tile_pool(name="tpps", bufs=1, space="PSUM") as tpps,
        ):
            # ---- static tiles ----
            wqk_sb = static.tile([128, 2, 512], bf16)
            wv_sb = static.tile([128, 2, 264], bf16)
            wo_sb = static.tile([128, 2, 256], bf16)
            bqk_sb = static.tile([128, 4], f32)
            bv_sb = static.tile([1, 264], bf16)
            bo_sb = static.tile([128, 2], f32)
            ones_sb = static.tile([1, 128], bf16)
            ident = static.tile([128, 128], bf16)

            nc.sync.dma_start(out=wqk_sb, in_=wqk_v)
            nc.sync.dma_start(out=wv_sb, in_=wv_v)
            nc.sync.dma_start(out=wo_sb, in_=wo_v)
            nc.sync.dma_start(out=bqk_sb, in_=bqk_d[:, :])
            nc.sync.dma_start(out=bv_sb, in_=bv_d[:, :])
            nc.sync.dma_start(out=bo_sb, in_=bo_d[:, :])
            nc.gpsimd.memset(ones_sb, 1.0)
            make_identity(nc, ident)

            # v-bias row (+ softmax ones columns), constant across all pairs:
            # bvx = ones^T (x) bv, computed once and reused as the vt evac addend
            bvx_sb = static.tile([128, 264], f32)
            bvx_ps = projps.tile([128, 264], f32, tag="proj")
            nc.tensor.matmul(
                out=bvx_ps, lhsT=ones_sb[:, :], rhs=bv_sb[:, :],
                start=True, stop=True,
            )
            nc.vector.tensor_copy(out=bvx_sb, in_=bvx_ps)

            def slab_body(i):
                # ---- load slab: [128, chunk, 8 rows, 224] ----
                x_sb = xpool.tile([128, 2, 8, 224], bf16)
                nc.gpsimd.dma_start(out=x_sb, in_=x_v[:, :, ds(i * 8, 8), :])

                # token-major copy (window-major tokens; windows of 8x8)
                x_tok = xpool.tile([128, 2, 1792], bf16, tag="xtok")
                for ch in range(2):
                    nc.vector.tensor_copy(
                        out=x_tok[:, ch],
                        in_=x_sb[:, ch].rearrange("p h (W w) -> p W h w", w=8),
                    )

                q_sb = qkpool.tile([128, 2, 1792], bf16, tag="q")
                k_sb = qkpool.tile([128, 2, 1792], bf16, tag="k")
                o_sb = opool.tile([128, 2, 1792], bf16)
                y_sb = ypool.tile([128, 2, 8, 224], bf16)

                # ---- Q,K projection, groups of 7 windows (448 tokens) ----
                for g in range(4):
                    for eb in range(4):
                        ps = projps.tile([128, 448], f32, tag="proj")
                        nc.tensor.matmul(
                            out=ps, lhsT=wqk_sb[:, 0, 128 * eb : 128 * eb + 128],
                            rhs=x_tok[:, 0, 448 * g : 448 * g + 448],
                            start=True, stop=False,
                        )
                        nc.tensor.matmul(
                            out=ps, lhsT=wqk_sb[:, 1, 128 * eb : 128 * eb + 128],
                            rhs=x_tok[:, 1, 448 * g : 448 * g + 448],
                            start=False, stop=True,
                        )
                        dest = (q_sb, q_sb, k_sb, k_sb)[eb]
                        dst = dest[:, eb % 2, 448 * g : 448 * g + 448]
                        if eb % 2 == 0:
                            nc.vector.tensor_scalar_add(
                                out=dst, in0=ps, scalar1=bqk_sb[:, eb : eb + 1]
                            )
                        else:
                            nc.scalar.activation(
                                out=dst, in_=ps, func=AF.Identity,
                                bias=bqk_sb[:, eb : eb + 1], scale=1.0,
                            )

                # ---- attention: 14 window pairs ----
                for pp in range(14):
                    # V^T projection for this pair (tokens on partitions)
                    vps = projps.tile([128, 264], f32, tag="proj")
                    nc.tensor.matmul(
                        out=vps, lhsT=x_tok[:, 0, 128 * pp : 128 * pp + 128],
                        rhs=wv_sb[:, 0, :], start=True, stop=False,
                    )
                    nc.tensor.matmul(
                        out=vps, lhsT=x_tok[:, 1, 128 * pp : 128 * pp + 128],
                        rhs=wv_sb[:, 1, :], start=False, stop=True,
                    )
                    vt = vtpool.tile([128, 264], bf16)
                    nc.vector.tensor_add(out=vt, in0=vps, in1=bvx_sb)

                    # logits^T for both windows at once (FWL-eligible 128-col
                    # stationary); cross-window quadrants are garbage that the
                    # AV slices below never read. Bank per j.
                    LT = ltps.tile([128, 4, 512], f32)
                    for h in range(8):
                        j, hi = h % 4, h // 4
                        nc.tensor.matmul(
                            out=LT[:, j, 128 * hi : 128 * hi + 128],
                            lhsT=k_sb[32 * j : 32 * j + 32, hi,
                                      128 * pp : 128 * pp + 128],
                            rhs=q_sb[32 * j : 32 * j + 32, hi,
                                     128 * pp : 128 * pp + 128],
                            start=True, stop=True,
                            tile_position=(32 * j, 0),
                        )

                    # PT = exp(scale * logits^T), bf16; col = 256j+128hi+64wi2+s
                    PT = ptpool.tile([128, 1024], bf16)
                    nc.scalar.activation(
                        out=PT[:].rearrange("p (j c) -> p j c", j=4),
                        in_=LT[:, :, 0:256], func=AF.Exp, scale=EXP_SCALE,
                    )

                    # AV: o^T[(wi,s), (h, d|sum)] with denominator col 33h+32
                    OT = otps.tile([128, 264], f32)
                    for h in range(8):
                        j, hi = h % 4, h // 4
                        for wi in range(2):
                            nc.tensor.matmul(
                                out=OT[64 * wi : 64 * wi + 64, 33 * h : 33 * h + 33],
                                lhsT=PT[64 * wi : 64 * wi + 64,
                                        256 * j + 128 * hi + 64 * wi :
                                        256 * j + 128 * hi + 64 * wi + 64],
                                rhs=vt[64 * wi : 64 * wi + 64, 33 * h : 33 * h + 33],
                                start=True, stop=True,
                                tile_position=(64 * wi, 64 * wi),
                            )

                    # normalize: oT_sb[:, 32h:32h+32] = OT[:, 33h:33h+32] / denom
                    rs = spool.tile([128, 8], f32, tag="rs")
                    denom = OT[:].rearrange("p (h c) -> p h c", c=33)[:, :, 32]
                    nc.vector.reciprocal(out=rs, in_=denom)
                    oT_sb = otnpool.tile([128, 256], bf16)
                    r = rs[:]
                    r_b = bass.AP(r.tensor, r.offset, [r.ap[0], [1, 8], [0, 32]])
                    nc.vector.tensor_mul(
                        out=oT_sb[:].rearrange("p (h c) -> p h c", c=32),
                        in0=OT[:].rearrange("p (h c) -> p h c", c=33)[:, :, 0:32],
                        in1=r_b,
                    )

                    # transpose back to channel-major o
                    tp = tpps.tile([128, 2, 128], bf16)
                    for ch in range(2):
                        nc.tensor.transpose(
                            out=tp[:, ch, :],
                            in_=oT_sb[:, 128 * ch : 128 * ch + 128],
                            identity=ident,
                        )
                    if pp % 2 == 0:
                        nc.scalar.copy(
                            out=o_sb[:, :, 128 * pp : 128 * pp + 128], in_=tp
                        )
                    else:
                        nc.vector.tensor_copy(
                            out=o_sb[:, :, 128 * pp : 128 * pp + 128], in_=tp
                        )

                # ---- out-projection ----
                for g in range(4):
                    yg = [
                        y_sb[:, ob].rearrange("p h (G j w) -> p G j h w", j=7, w=8)[:, g]
                        for ob in range(2)
                    ]
                    for ob in range(2):
                        ps = projps.tile([128, 448], f32, tag="proj")
                        nc.tensor.matmul(
                            out=ps, lhsT=wo_sb[:, 0, 128 * ob : 128 * ob + 128],
                            rhs=o_sb[:, 0, 448 * g : 448 * g + 448],
                            start=True, stop=False,
                        )
                        nc.tensor.matmul(
                            out=ps, lhsT=wo_sb[:, 1, 128 * ob : 128 * ob + 128],
                            rhs=o_sb[:, 1, 448 * g : 448 * g + 448],
                            start=False, stop=True,
                        )
                        psv = ps[:].rearrange("p (j h w) -> p j h w", h=8, w=8)
                        if (g + ob) % 2 == 0:
                            nc.vector.tensor_scalar_add(
                                out=yg[ob], in0=psv, scalar1=bo_sb[:, ob : ob + 1]
                            )
                        else:
                            nc.scalar.activation(
                                out=yg[ob], in_=psv, func=AF.Identity,
                                bias=bo_sb[:, ob : ob + 1], scale=1.0,
                            )

                nc.gpsimd.dma_start(out=y_v[:, :, ds(i * 8, 8), :], in_=y_sb)

            for i in range(n_slabs):
                slab_body(i)

    if split_waits:
        _split_excess_waits(nc)
    return nc


def _split_excess_waits(nc, limit=1):
    import concourse.mybir as mybir

    n_new = 0
    for f in nc.m.functions:
        for bb in f.blocks:
            insts = bb.instructions
            i = 0
            while i < len(insts):
                inst = insts[i]
                si = inst.sync_info
                if si is not None and si.on_wait and len(si.on_wait) > limit:
                    waits = list(si.on_wait)
                    si.on_wait = waits[:limit]
                    rest = waits[limit:]
                    for k in range(0, len(rest), limit):
                        nop = mybir.InstNoOp(name=f"{inst.name}-wsplit{k}", ins=[], outs=[])
                        nop.engine = inst.engine
                        nop.sync_info = mybir.SyncInfo(on_wait=rest[k : k + limit], on_update=[])
                        insts.insert(i, nop)
                        n_new += 1
                        i += 1
                i += 1
    return n_new


def _get_nc(n_slabs):
    if n_slabs not in _CACHE:
        _CACHE[n_slabs] = _build(n_slabs)
    return _CACHE[n_slabs]


def _host_prep(w_in, b_in, w_out, b_out):
    bf = ml_dtypes.bfloat16
    f = np.float32
    w_in = np.asarray(w_in, f)
    b_in = np.asarray(b_in, f)
    w_out = np.asarray(w_out, f)
    b_out = np.asarray(b_out, f)
    wqkT = np.ascontiguousarray(w_in[0:512].T).astype(bf)          # [256, 512]
    wv = w_in[512:768]                                             # [256 vc, 256 cin]
    wv_aug = np.zeros((256, 264), f)
    bv_aug = np.zeros((1, 264), f)
    for h in range(8):
        wv_aug[:, 33 * h : 33 * h + 32] = wv[32 * h : 32 * h + 32, :].T
        bv_aug[0, 33 * h : 33 * h + 32] = b_in[512 + 32 * h : 512 + 32 * h + 32]
        bv_aug[0, 33 * h + 32] = 1.0
    wvT = wv_aug.astype(bf)
    bvT = bv_aug.astype(bf)
    woT = np.ascontiguousarray(w_out.T).astype(bf)                 # [256, 256]
    bqk = np.ascontiguousarray(b_in[0:512].reshape(4, 128).T)      # [128, 4]
    bo = np.ascontiguousarray(b_out.reshape(2, 128).T)             # [128, 2]
    return wqkT, wvT, bvT, woT, bqk, bo


def _bf16_to_f32(a):
    return np.asarray(a).astype(np.float32)


def kernel(x, w_in, b_in, w_out, b_out, _n_slabs=N_SLABS, _trace=False):
    if _trace:
        _install_ntff_hook()
    from concourse.bass_utils import run_bass_kernel_spmd

    x = np.asarray(x)
    B = x.shape[0]
    x16 = x.astype(ml_dtypes.bfloat16)
    wqkT, wvT, bvT, woT, bqk, bo = _host_prep(w_in, b_in, w_out, b_out)
    nc = _get_nc(_n_slabs)
    in_maps = []
    for b in range(CORES):
        in_maps.append({
            "x": x16[b % B],
            "wqkT": wqkT, "wvT": wvT, "woT": woT,
            "bqk": bqk, "bv": bvT, "bo": bo,
        })
    res = run_bass_kernel_spmd(
        nc, in_maps, core_ids=list(range(CORES)), trace=_trace
    )
    y = np.empty(x.shape, np.float32)
    for b in range(B):
        y[b] = _bf16_to_f32(res.results[b]["y"])
    kernel.last_result = res
    kernel.total_exec_time_ns = res.exec_time_ns
    return y


# revision 5
# speedup vs baseline: 17056.1276x; 1.0662x over previous
"""LocalMHSA2D Trainium2 kernel v2: window (8x8) multi-head self-attention.

Full inputs -> shard batch B=8 across 8 NeuronCores -> full output.

Per-core dataflow (x_b: [256, 224, 224] bf16, channels-first), one hardware
loop over 28 slabs of 8 pixel rows (28 windows each, 14 window pairs):

  - Q,K projection channel-major (contraction over C on partitions), bf16.
  - V projected directly into transposed layout (tokens on partitions) by
    swapping matmul operands: lhsT = x-tile [cin, 128 tok], rhs = w_v.
    w_v is augmented to 264 columns: per head 32 value dims + 1 ones column
    (via the bias row), which later yields softmax denominators for free.
  - Attention per window pair, all layouts chosen so no P/V transposes are
    needed:
      logits^T directly via lhsT=k, rhs=q -> PSUM [128 (wi,t), (j,hi,s)];
      PT = exp(scale*logits^T) on ACT -> SBUF bf16;
      AV: lhsT = PT slice [t, s], rhs = vt [t, 33] -> o^T [(wi,s), (h, d|sum)]
      with the softmax denominator interleaved as column 33h+32;
      normalize o^T with 8 per-head tensor_scalar_mul (factor = per-partition
      reciprocal of the denominator column);
      2 PE transposes (128x128) restore channel-major o for the out-proj.
  - Out-projection + bias, written back in spatial order, y stored as bf16.

This walrus build rejects instructions carrying >1 semaphore wait
("Too many sync wait commands"), so a post-pass splits excess waits
onto same-engine no-ops.
"""

import numpy as np
import ml_dtypes

CORES = 8
N_SLABS = 28

_CACHE = {}


def _install_ntff_hook():
    """Register the axon NTFF profile hook if the image's antenv lacks it.

    concourse.bass_utils fetches the hook via ``antenv.axon_hooks``; on this
    image that module is absent, so ``trace=True`` can't produce a device
    exec time. The boot module ships the ctypes-based hook constructor, so
    provide the missing registry in-process.
    """
    import sys
    try:
        from antenv.axon_hooks import get_axon_ntff_profile_hook  # noqa: F401
        return
    except ImportError:
        pass
    try:
        import types
        from trn_agent_boot.trn_boot import _ntff_profile_via_ctypes
        hook = _ntff_profile_via_ctypes("/opt/axon/libaxon_pjrt.so")
        if hook is None:
            return
        mod = types.ModuleType("antenv.axon_hooks")
        _h = [hook]
        mod.set_axon_ntff_profile_hook = lambda h: _h.__setitem__(0, h)
        mod.get_axon_ntff_profile_hook = lambda: _h[0]
        sys.modules["antenv.axon_hooks"] = mod
    except Exception:
        pass


def _build(n_slabs, split_waits=True):
    import concourse.bass as bass
    import concourse.mybir as mybir
    import concourse.tile as tile
    from concourse.masks import make_identity
    from concourse.bass import ds

    f32 = mybir.dt.float32
    bf16 = mybir.dt.bfloat16
    AF = mybir.ActivationFunctionType

    nc = bass.Bass()
    HH = n_slabs * 8
    x_d = nc.dram_tensor("x", [256, HH, 224], bf16, kind="ExternalInput")
    wqk_d = nc.dram_tensor("wqkT", [256, 512], bf16, kind="ExternalInput")
    wv_d = nc.dram_tensor("wvT", [256, 264], bf16, kind="ExternalInput")
    wo_d = nc.dram_tensor("woT", [256, 256], bf16, kind="ExternalInput")
    bqk_d = nc.dram_tensor("bqk", [128, 4], f32, kind="ExternalInput")
    bv_d = nc.dram_tensor("bv", [1, 264], bf16, kind="ExternalInput")
    bo_d = nc.dram_tensor("bo", [128, 2], f32, kind="ExternalInput")
    y_d = nc.dram_tensor("y", [256, HH, 224], bf16, kind="ExternalOutput")

    x_v = x_d.rearrange("(cc p) hh w -> p cc hh w", p=128)
    y_v = y_d.rearrange("(cc p) hh w -> p cc hh w", p=128)
    wqk_v = wqk_d.rearrange("(cc p) e -> p cc e", p=128)
    wv_v = wv_d.rearrange("(cc p) e -> p cc e", p=128)
    wo_v = wo_d.rearrange("(cc p) e -> p cc e", p=128)

    EXP_SCALE = float(1.0 / np.sqrt(32.0))

    with tile.TileContext(nc) as tc:
        with (
            tc.tile_pool(name="static", bufs=1) as static,
            tc.tile_pool(name="xin", bufs=2) as xpool,
            tc.tile_pool(name="qk", bufs=2) as qkpool,
            tc.tile_pool(name="vt", bufs=6) as vtpool,
            tc.tile_pool(name="pt", bufs=4) as ptpool,
            tc.tile_pool(name="otn", bufs=4) as otnpool,
            tc.tile_pool(name="osb", bufs=2) as opool,
            tc.tile_pool(name="ysb", bufs=2) as ypool,
            tc.tile_pool(name="small", bufs=4) as spool,
            tc.tile_pool(name="projps", bufs=2, space="PSUM") as projps,
            tc.tile_pool(name="ltps", bufs=1, space="PSUM") as ltps,
            tc.tile_pool(name="otps", bufs=1, space="PSUM") as otps,
            tc.tile_pool(name="tpps", bufs=1, space="PSUM") as tpps,
        ):
            # ---- static tiles ----
            wqk_sb = static.tile([128, 2, 512], bf16)
            wv_sb = static.tile([128, 2, 264], bf16)
            wo_sb = static.tile([128, 2, 256], bf16)
            bqk_sb = static.tile([128, 4], f32)
            bv_sb = static.tile([1, 264], bf16)
            bo_sb = static.tile([128, 2], f32)
            ones_sb = static.tile([1, 128], bf16)
            ident = static.tile([128, 128], bf16)

            nc.sync.dma_start(out=wqk_sb, in_=wqk_v)
            nc.sync.dma_start(out=wv_sb, in_=wv_v)
            nc.sync.dma_start(out=wo_sb, in_=wo_v)
            nc.sync.dma_start(out=bqk_sb, in_=bqk_d[:, :])
            nc.sync.dma_start(out=bv_sb, in_=bv_d[:, :])
            nc.sync.dma_start(out=bo_sb, in_=bo_d[:, :])
            nc.gpsimd.memset(ones_sb, 1.0)
            make_identity(nc, ident)

            # v-bias row (+ softmax ones columns), constant across all pairs:
            # bvx = ones^T (x) bv, computed once and reused as the vt evac addend
            bvx_sb = static.tile([128, 264], f32)
            bvx_ps = projps.tile([128, 264], f32, tag="proj")
            nc.tensor.matmul(
                out=bvx_ps, lhsT=ones_sb[:, :], rhs=bv_sb[:, :],
                start=True, stop=True,
            )
            nc.vector.tensor_copy(out=bvx_sb, in_=bvx_ps)

            def slab_body(i):
                # ---- load slab: [128, chunk, 8 rows, 224] ----
                x_sb = xpool.tile([128, 2, 8, 224], bf16)
                nc.gpsimd.dma_start(out=x_sb, in_=x_v[:, :, ds(i * 8, 8), :])

                # token-major copy (window-major tokens; windows of 8x8)
                x_tok = xpool.tile([128, 2, 1792], bf16, tag="xtok")
                for ch in range(2):
                    nc.vector.tensor_copy(
                        out=x_tok[:, ch],
                        in_=x_sb[:, ch].rearrange("p h (W w) -> p W h w", w=8),
                    )

                q_sb = qkpool.tile([128, 2, 1792], bf16, tag="q")
                k_sb = qkpool.tile([128, 2, 1792], bf16, tag="k")
                o_sb = opool.tile([128, 2, 1792], bf16)
                y_sb = ypool.tile([128, 2, 8, 224], bf16)

                # ---- Q,K projection, groups of 7 windows (448 tokens) ----
                for g in range(4):
                    for eb in range(4):
                        ps = projps.tile([128, 448], f32, tag="proj")
                        nc.tensor.matmul(
                            out=ps, lhsT=wqk_sb[:, 0, 128 * eb : 128 * eb + 128],
                            rhs=x_tok[:, 0, 448 * g : 448 * g + 448],
                            start=True, stop=False,
                        )
                        nc.tensor.matmul(
                            out=ps, lhsT=wqk_sb[:, 1, 128 * eb : 128 * eb + 128],
                            rhs=x_tok[:, 1, 448 * g : 448 * g + 448],
                            start=False, stop=True,
                        )
                        dest = (q_sb, q_sb, k_sb, k_sb)[eb]
                        dst = dest[:, eb % 2, 448 * g : 448 * g + 448]
                        if eb % 2 == 0:
                            nc.vector.tensor_scalar_add(
                                out=dst, in0=ps, scalar1=bqk_sb[:, eb : eb + 1]
                            )
                        else:
                            nc.scalar.activation(
                                out=dst, in_=ps, func=AF.Identity,
                                bias=bqk_sb[:, eb : eb + 1], scale=1.0,
                            )

                # ---- attention: 14 window pairs ----
                for pp in range(14):
                    # V^T projection for this pair (tokens on partitions)
                    vps = projps.tile([128, 264], f32, tag="proj")
                    nc.tensor.matmul(
                        out=vps, lhsT=x_tok[:, 0, 128 * pp : 128 * pp + 128],
                        rhs=wv_sb[:, 0, :], start=True, stop=False,
                    )
                    nc.tensor.matmul(
                        out=vps, lhsT=x_tok[:, 1, 128 * pp : 128 * pp + 128],
                        rhs=wv_sb[:, 1, :], start=False, stop=True,
                    )
                    vt = vtpool.tile([128, 264], bf16)
                    nc.vector.tensor_add(out=vt, in0=vps, in1=bvx_sb)

                    # logits^T for both windows at once (FWL-eligible 128-col
                    # stationary); cross-window quadrants are garbage that the
                    # AV slices below never read. Bank per j.
                    LT = ltps.tile([128, 4, 512], f32)
                    for h in range(8):
                        j, hi = h % 4, h // 4
                        nc.tensor.matmul(
                            out=LT[:, j, 128 * hi : 128 * hi + 128],
                            lhsT=k_sb[32 * j : 32 * j + 32, hi,
                                      128 * pp : 128 * pp + 128],
                            rhs=q_sb[32 * j : 32 * j + 32, hi,
                                     128 * pp : 128 * pp + 128],
                            start=True, stop=True,
                            tile_position=(32 * j, 0),
                        )

                    # PT = exp(scale * logits^T), bf16; col = 256j+128hi+64wi2+s
                    PT = ptpool.tile([128, 1024], bf16)
                    nc.scalar.activation(
                        out=PT[:].rearrange("p (j c) -> p j c", j=4),
                        in_=LT[:, :, 0:256], func=AF.Exp, scale=EXP_SCALE,
                    )

                    # AV: o^T[(wi,s), (h, d|sum)] with denominator col 33h+32
                    OT = otps.tile([128, 264], f32)
                    for h in range(8):
                        j, hi = h % 4, h // 4
                        for wi in range(2):
                            nc.tensor.matmul(
                                out=OT[64 * wi : 64 * wi + 64, 33 * h : 33 * h + 33],
                                lhsT=PT[64 * wi : 64 * wi + 64,
                                        256 * j + 128 * hi + 64 * wi :
                                        256 * j + 128 * hi + 64 * wi + 64],
                                rhs=vt[64 * wi : 64 * wi + 64, 33 * h : 33 * h + 33],
                                start=True, stop=True,
                                tile_position=(64 * wi, 64 * wi),
                            )

                    # normalize: oT_sb[:, 32h:32h+32] = OT[:, 33h:33h+32] / denom
                    rs = spool.tile([128, 8], f32, tag="rs")
                    denom = OT[:].rearrange("p (h c) -> p h c", c=33)[:, :, 32]
                    nc.vector.reciprocal(out=rs, in_=denom)
                    oT_sb = otnpool.tile([128, 256], bf16)
                    r = rs[:]
                    r_b = bass.AP(r.tensor, r.offset, [r.ap[0], [1, 8], [0, 32]])
                    nc.vector.tensor_mul(
                        out=oT_sb[:].rearrange("p (h c) -> p h c", c=32),
                        in0=OT[:].rearrange("p (h c) -> p h c", c=33)[:, :, 0:32],
                        in1=r_b,
                    )

                    # transpose back to channel-major o
                    tp = tpps.tile([128, 2, 128], bf16)
                    for ch in range(2):
                        nc.tensor.transpose(
                            out=tp[:, ch, :],
                            in_=oT_sb[:, 128 * ch : 128 * ch + 128],
                            identity=ident,
                        )
                    if pp % 2 == 0:
                        nc.scalar.copy(
                            out=o_sb[:, :, 128 * pp : 128 * pp + 128], in_=tp
                        )
                    else:
                        nc.vector.tensor_copy(
                            out=o_sb[:, :, 128 * pp : 128 * pp + 128], in_=tp
                        )

                # ---- out-projection ----
                for g in range(4):
                    yg = [
                        y_sb[:, ob].rearrange("p h (G j w) -> p G j h w", j=7, w=8)[:, g]
                        for ob in range(2)
                    ]
                    for ob in range(2):
                        ps = projps.tile([128, 448], f32, tag="proj")
                        nc.tensor.matmul(
                            out=ps, lhsT=wo_sb[:, 0, 128 * ob : 128 * ob + 128],
                            rhs=o_sb[:, 0, 448 * g : 448 * g + 448],
                            start=True, stop=False,
                        )
                        nc.tensor.matmul(
                            out=ps, lhsT=wo_sb[:, 1, 128 * ob : 128 * ob + 128],
                            rhs=o_sb[:, 1, 448 * g : 448 * g + 448],
                            start=False, stop=True,
                        )
                        psv = ps[:].rearrange("p (j h w) -> p j h w", h=8, w=8)
                        if (g + ob) % 2 == 0:
                            nc.vector.tensor_scalar_add(
                                out=yg[ob], in0=psv, scalar1=bo_sb[:, ob : ob + 1]
                            )
                        else:
                            nc.scalar.activation(
                                out=yg[ob], in_=psv, func=AF.Identity,
                                bias=bo_sb[:, ob : ob + 1], scale=1.0,
                            )

                nc.sync.dma_start(out=y_v[:, :, ds(i * 8, 8), :], in_=y_sb)

            for i in range(n_slabs):
                slab_body(i)

    if split_waits:
        _split_excess_waits(nc)
    return nc


def _split_excess_waits(nc, limit=1):
    import concourse.mybir as mybir

    n_new = 0
    for f in nc.m.functions:
        for bb in f.blocks:
            insts = bb.instructions
            i = 0
            while i < len(insts):
                inst = insts[i]
                si = inst.sync_info
                if si is not None and si.on_wait and len(si.on_wait) > limit:
                    waits = list(si.on_wait)
                    si.on_wait = waits[:limit]
                    rest = waits[limit:]
                    for k in range(0, len(rest), limit):
                        nop = mybir.InstNoOp(name=f"{inst.name}-wsplit{k}", ins=[], outs=[])
                        nop.engine = inst.engine
                        nop.sync_info = mybir.SyncInfo(on_wait=rest[k : k + limit], on_update=[])
                        insts.insert(i, nop)
                        n_new += 1
                        i += 1
                i += 1
    return n_new


def _get_nc(n_slabs):
    if n_slabs not in _CACHE:
        _CACHE[n_slabs] = _build(n_slabs)
    return _CACHE[n_slabs]


def _host_prep(w_in, b_in, w_out, b_out):
    bf = ml_dtypes.bfloat16
    f = np.float32
    w_in = np.asarray(w_in, f)
    b_in = np.asarray(b_in, f)
    w_out = np.asarray(w_out, f)
    b_out = np.asarray(b_out, f)
    wqkT = np.ascontiguousarray(w_in[0:512].T).astype(bf)          # [256, 512]
    wv = w_in[512:768]                                             # [256 vc, 256 cin]
    wv_aug = np.zeros((256, 264), f)
    bv_aug = np.zeros((1, 264), f)
    for h in range(8):
        wv_aug[:, 33 * h : 33 * h + 32] = wv[32 * h : 32 * h + 32, :].T
        bv_aug[0, 33 * h : 33 * h + 32] = b_in[512 + 32 * h : 512 + 32 * h + 32]
        bv_aug[0, 33 * h + 32] = 1.0
    wvT = wv_aug.astype(bf)
    bvT = bv_aug.astype(bf)
    woT = np.ascontiguousarray(w_out.T).astype(bf)                 # [256, 256]
    bqk = np.ascontiguousarray(b_in[0:512].reshape(4, 128).T)      # [128, 4]
    bo = np.ascontiguousarray(b_out.reshape(2, 128).T)             # [128, 2]
    return wqkT, wvT, bvT, woT, bqk, bo


def _bf16_to_f32(a):
    return np.asarray(a).astype(np.float32)


def kernel(x, w_in, b_in, w_out, b_out, _n_slabs=N_SLABS, _trace=False):
    if _trace:
        _install_ntff_hook()
    from concourse.bass_utils import run_bass_kernel_spmd

    x = np.asarray(x)
    B = x.shape[0]
    x16 = x.astype(ml_dtypes.bfloat16)
    wqkT, wvT, bvT, woT, bqk, bo = _host_prep(w_in, b_in, w_out, b_out)
    nc = _get_nc(_n_slabs)
    in_maps = []
    for b in range(CORES):
        in_maps.append({
            "x": x16[b % B],
            "wqkT": wqkT, "wvT": wvT, "woT": woT,
            "bqk": bqk, "bv": bvT, "bo": bo,
        })
    res = run_bass_kernel_spmd(
        nc, in_maps, core_ids=list(range(CORES)), trace=_trace
    )
    y = np.empty(x.shape, np.float32)
    for b in range(B):
        y[b] = _bf16_to_f32(res.results[b]["y"])
    kernel.last_result = res
    kernel.total_exec_time_ns = res.exec_time_ns
    return y
